# revision 20
# baseline (speedup 1.0000x reference)
"""GraphTransformer (2x PyG TransformerConv + linear) on 8 trn2 NeuronCores.

v3: window-structured edge blocks, PE-side alpha, phi-as-stationary agg.

Per-core geometry: iteration = 16 subtiles x 128 edge slots; 8 windows of
16 dst slots each; window w's even-src-parity edges fill subtile w, odd
fill subtile 8+w.  Gather indices are pair ids (src>>1, < 25000) so one
int16 table covers all nodes (even/odd selected by a 64/128-element offset
into 256/512-byte pair rows via elem_step).

Per subtile: PE transposes the gathered x chunk (edge-major -> feature-
major), computes all-pairs alpha = xs @ uT_window [128e x (16 slots * H)],
scalar exps it, DVE masks with an is_equal one-hot (winid vs col pattern)
giving phi, and the aggregation matmul uses phi as the stationary operand
with rhs [x | 1] (ones column written over an unused gather column),
yielding per-(slot, head) numerator and denominator in one PSUM region.
Normalization, Wv application, skip+relu and the next layer's projection
are fused into the same loop (proj/uT/q2T SBUF-resident, no staging
round-trips, no host-built one-hot matrices).
"""
import sys

sys.path.insert(0, "/opt/trn_rl_repo")
import numpy as np
import ml_dtypes
import concourse.bass as bass
import concourse.bacc as bacc
import concourse.tile as tile
from concourse import mybir
from concourse import library_config
from concourse.bass_utils import run_bass_kernel_spmd

F32 = mybir.dt.float32
BF16 = mybir.dt.bfloat16
I16 = mybir.dt.int16
BF = ml_dtypes.bfloat16

NCORES = 8
NNODE = 50000
NPAIR = 25088            # padded pair-table rows
NW = 8                   # windows per iteration
WS = 16                  # dst slots per window
WCAP = 128               # per-parity edge capacity per window

_built = {}


def _bc(ap, p):
    return bass.AP(tensor=ap.tensor, offset=ap.offset, ap=[[0, p]] + list(ap.ap[1:]))


def _off(ap, off):
    """Offset an ap and drop its last row so it stays in bounds."""
    a = [list(p) for p in ap.ap]
    a[0][1] -= 1
    return bass.AP(tensor=ap.tensor, offset=ap.offset + off, ap=a)


def _build_L1(B2):
    NS = B2 * 128
    nc = bacc.Bacc(num_swdge_queues=4)
    xT = nc.declare_dram_parameter("xT", [64, NS], BF16, isOutput=False)
    W1 = nc.declare_dram_parameter("W1", [64, 512], BF16, isOutput=False)
    b1 = nc.declare_dram_parameter("b1", [1, 512], F32, isOutput=False)
    tab = nc.declare_dram_parameter("tab", [NPAIR, 128], BF16, isOutput=False)
    idx = nc.declare_dram_parameter("idx", [B2, 128, 128], I16, isOutput=False)
    win = nc.declare_dram_parameter("win", [B2, 128, 16], BF16, isOutput=False)
    cpat = nc.declare_dram_parameter("cpat", [128, 64], BF16, isOutput=False)
    Wv = nc.declare_dram_parameter("Wv", [64, 256], BF16, isOutput=False)
    W2 = nc.declare_dram_parameter("W2", [256, 256], BF16, isOutput=False)
    b2 = nc.declare_dram_parameter("b2", [1, 256], F32, isOutput=False)
    idn = nc.declare_dram_parameter("idn", [128, 128], BF16, isOutput=False)
    outt = nc.declare_dram_parameter("outt", [NS, 256], BF16, isOutput=True)

    with tile.TileContext(nc) as tc:
        nc.gpsimd.load_library(library_config.mlp)
        with tc.tile_pool(name="one", bufs=1) as one:
            W1t = one.tile([64, 512], BF16)
            nc.sync.dma_start(out=W1t[:], in_=W1[:])
            b1t = one.tile([128, 512], F32)
            nc.sync.dma_start(out=b1t[:], in_=_bc(b1[:], 128))
            Wvt = one.tile([64, 256], BF16)
            nc.sync.dma_start(out=Wvt[:], in_=Wv[:])
            W2t = one.tile([128, 2, 256], BF16)
            for k in range(2):
                nc.sync.dma_start(out=W2t[:, k, :], in_=W2[k * 128:(k + 1) * 128, :])
            b2t = one.tile([128, 256], F32)
            nc.sync.dma_start(out=b2t[:], in_=_bc(b2[:], 128))
            ident = one.tile([128, 128], BF16)
            nc.sync.dma_start(out=ident[:], in_=idn[:])
            cpt = one.tile([128, 64], BF16)
            nc.sync.dma_start(out=cpt[:], in_=cpat[:])
            uTall = one.tile([64, B2, 4, 128], BF16, name="uTall")
            projS = one.tile([128, B2, 256], BF16, name="projS")

            # ---- phase P: proj; u transposed into uTall, sk into projS ----
            with tc.tile_pool(name="psb", bufs=3) as sb, \
                 tc.tile_pool(name="pps", bufs=2, space="PSUM") as ps:
                for i in range(B2):
                    xt = sb.tile([64, 128], BF16, tag="xt")
                    nc.sync.dma_start(out=xt[:], in_=xT[:, i * 128:(i + 1) * 128])
                    pp = ps.tile([128, 512], F32, tag="pp")
                    nc.tensor.matmul(out=pp[:], lhsT=xt[:], rhs=W1t[:],
                                     start=True, stop=True)
                    nc.vector.tensor_add(projS[:, i, :], pp[:, 256:512],
                                         b1t[:, 256:512])
                    ut = sb.tile([128, 256], BF16, tag="ut")
                    nc.vector.tensor_add(ut[:], pp[:, 0:256], b1t[:, 0:256])
                    uTp = ps.tile([128, 2, 512], BF16, tag="uTp")
                    for k in range(2):
                        nc.tensor.transpose(out=uTp[:, k, 0:128],
                                            in_=ut[:, k * 128:(k + 1) * 128],
                                            identity=ident[:])
                    for h in range(4):
                        nc.scalar.activation(
                            uTall[:, i, h, :],
                            uTp[(h % 2) * 64:(h % 2) * 64 + 64, h // 2, 0:128],
                            mybir.ActivationFunctionType.Copy)

            # ---- block loop (conv1 + dense fused) ----
            with tc.tile_pool(name="sb", bufs=3) as sb, \
                 tc.tile_pool(name="sb2", bufs=2) as sb2, \
                 tc.tile_pool(name="tps", bufs=1, space="PSUM") as tps, \
                 tc.tile_pool(name="qps", bufs=1, space="PSUM") as qps, \
                 tc.tile_pool(name="aps", bufs=2, space="PSUM") as aps, \
                 tc.tile_pool(name="cps", bufs=1, space="PSUM") as cps, \
                 tc.tile_pool(name="dps", bufs=1, space="PSUM") as dps:
                for i in range(B2):
                    ix = sb.tile([128, 128], I16, tag="ix")
                    nc.sync.dma_start(out=ix[:], in_=idx[i])
                    wt = sb.tile([128, 16], BF16, tag="wt")
                    nc.sync.dma_start(out=wt[:], in_=win[i])
                    xs = sb.tile([128, 16, 128], BF16, tag="xs")
                    nc.gpsimd.dma_gather(
                        xs[:, 0:8, :], tab[:], ix[:, 0:64], 1024, 1024,
                        128, single_packet=False, queue_num=(2 * i) % 4)
                    nc.gpsimd.dma_gather(
                        xs[:, 8:16, :], _off(tab[:], 64), ix[:, 64:128],
                        1024, 1024, 128, single_packet=False,
                        queue_num=(2 * i + 1) % 4)
                    nc.vector.memset(xs[:, :, 64:65], 1.0)

                    agg = aps.tile([128, 4, 128], F32, tag="agg")
                    for half in range(2):
                        xsTp = tps.tile([64, 8, 128], BF16, tag="xsTp")
                        for k in range(8):
                            t = half * 8 + k
                            nc.tensor.transpose(out=xsTp[:, k, :],
                                                in_=xs[:, t, 0:64],
                                                identity=ident[:])
                        xsT = sb2.tile([64, 8, 128], BF16, tag="xsT")
                        nc.scalar.activation(xsT[:], xsTp[:],
                                             mybir.ActivationFunctionType.Copy)
                        alp = qps.tile([128, 8, 64], F32, tag="alp")
                        for k in range(8):
                            nc.tensor.matmul(
                                out=alp[:, k, :], lhsT=xsT[:, k, :],
                                rhs=uTall[:, i, :, k * WS:(k + 1) * WS],
                                start=True, stop=True)
                        E = sb2.tile([128, 8, 64], BF16, tag="E")
                        nc.scalar.activation(E[:], alp[:],
                                             mybir.ActivationFunctionType.Exp,
                                             scale=0.125)
                        M01 = sb2.tile([128, 8, 64], BF16, tag="M01")
                        nc.vector.tensor_tensor(
                            out=M01[:],
                            in0=wt[:, half * 8:(half + 1) * 8]
                                .unsqueeze(2).to_broadcast([128, 8, 64]),
                            in1=cpt[:].unsqueeze(1)
                                .to_broadcast([128, 8, 64]),
                            op=mybir.AluOpType.is_equal)
                        phi = sb2.tile([128, 8, 64], BF16, tag="phi")
                        nc.vector.tensor_mul(phi[:], E[:], M01[:])
                        for k in range(8):
                            t = half * 8 + k
                            nc.tensor.matmul(
                                out=agg[64 * (k % 2):64 * (k % 2) + 64,
                                        k // 2, 0:65],
                                lhsT=phi[:, k, :], rhs=xs[:, t, 0:65],
                                start=(half == 0 and k <= 1),
                                stop=(half == 1 and k >= 6),
                                skip_group_check=True)

                    sc = sb.tile([128, 4], F32, tag="sc")
                    nc.vector.tensor_scalar_max(sc[:], agg[:, :, 64], 1e-30)
                    rs = sb.tile([128, 4], F32, tag="rs")
                    nc.vector.reciprocal(rs[:], sc[:])
                    zn = sb.tile([128, 4, 64], BF16, tag="zn")
                    nc.vector.tensor_mul(zn[:], agg[:, :, 0:64],
                                         rs[:].unsqueeze(2)
                                         .to_broadcast([128, 4, 64]))
                    znB = sb.tile([64, 4, 64], BF16, tag="znB")
                    nc.vector.tensor_copy(znB[:], zn[64:128, :, :])
                    znTp = cps.tile([64, 16, 4, 16], BF16, tag="znTp")
                    for w in range(8):
                        src = zn[0:64, w // 2, :] if w % 2 == 0 \
                            else znB[:, w // 2, :]
                        nc.tensor.transpose(out=znTp[:, w, :, :], in_=src,
                                            identity=ident[0:64, 0:64])
                    znT = sb.tile([64, 4, 8, 16], BF16, tag="znT")
                    nc.vector.tensor_copy(
                        znT[:].rearrange("p h w s -> p w h s"),
                        znTp[:, 0:8, :, :])
                    o1p = dps.tile([128, 512], F32, tag="o1p")
                    for h in range(4):
                        nc.tensor.matmul(out=o1p[:, h * 64:(h + 1) * 64],
                                         lhsT=znT[:, h, :, :],
                                         rhs=Wvt[:, h * 64:(h + 1) * 64],
                                         start=True, stop=True)
                    hb = sb.tile([128, 256], BF16, tag="hb")
                    nc.vector.tensor_add(hb[:], o1p[:, 0:256], projS[:, i, :])
                    nc.scalar.activation(hb[:], hb[:],
                                         mybir.ActivationFunctionType.Relu)
                    hTp = cps.tile([128, 2, 512], BF16, tag="hTp")
                    for k in range(2):
                        nc.tensor.transpose(out=hTp[:, k, 0:128],
                                            in_=hb[:, k * 128:(k + 1) * 128],
                                            identity=ident[:])
                    hT = sb.tile([128, 2, 128], BF16, tag="hT")
                    nc.vector.tensor_copy(hT[:], hTp[:, :, 0:128])
                    o2p = dps.tile([128, 512], F32, tag="o2p")
                    for k in range(2):
                        nc.tensor.matmul(out=o2p[:, 0:256], lhsT=hT[:, k, :],
                                         rhs=W2t[:, k, :],
                                         start=(k == 0), stop=(k == 1))
                    ot = sb.tile([128, 256], BF16, tag="ot")
                    nc.vector.tensor_add(ot[:], o2p[:, 0:256], b2t[:])
                    nc.sync.dma_start(out=outt[i * 128:(i + 1) * 128, :],
                                      in_=ot[:])
    nc.finalize()
    return nc


def _build_L2(B2):
    NS = B2 * 128
    nc = bacc.Bacc(num_swdge_queues=4)
    p2 = nc.declare_dram_parameter("p2", [NS, 128], BF16, isOutput=False)
    tab = nc.declare_dram_parameter("tab", [NPAIR, 256], BF16, isOutput=False)
    idx = nc.declare_dram_parameter("idx", [B2, 128, 128], I16, isOutput=False)
    win = nc.declare_dram_parameter("win", [B2, 128, 16], BF16, isOutput=False)
    cpat = nc.declare_dram_parameter("cpat", [128, 32], BF16, isOutput=False)
    Wl = nc.declare_dram_parameter("Wl", [64, 2], BF16, isOutput=False)
    bl = nc.declare_dram_parameter("bl", [1, 2], F32, isOutput=False)
    idn = nc.declare_dram_parameter("idn", [128, 128], BF16, isOutput=False)
    outf = nc.declare_dram_parameter("outf", [NS, 2], F32, isOutput=True)

    with tile.TileContext(nc) as tc:
        nc.gpsimd.load_library(library_config.mlp)
        with tc.tile_pool(name="one", bufs=1) as one:
            Wlt = one.tile([64, 2], BF16)
            nc.sync.dma_start(out=Wlt[:], in_=Wl[:])
            blt = one.tile([128, 2], F32)
            nc.sync.dma_start(out=blt[:], in_=_bc(bl[:], 128))
            ident = one.tile([128, 128], BF16)
            nc.sync.dma_start(out=ident[:], in_=idn[:])
            cpt = one.tile([128, 32], BF16)
            nc.sync.dma_start(out=cpt[:], in_=cpat[:])
            ones1 = one.tile([128, 1], BF16)
            nc.vector.memset(ones1[:], 1.0)
            q2T = one.tile([64, B2, 128], BF16, name="q2T")
            sk2S = one.tile([128, B2, 64], BF16, name="sk2S")

            # ---- phase P2: transpose q2, stash sk2 ----
            with tc.tile_pool(name="psb", bufs=3) as sb, \
                 tc.tile_pool(name="pps", bufs=2, space="PSUM") as ps:
                for i in range(B2):
                    pt = sb.tile([128, 128], BF16, tag="pt")
                    nc.sync.dma_start(out=pt[:],
                                      in_=p2[i * 128:(i + 1) * 128, :])
                    nc.vector.tensor_copy(sk2S[:, i, :], pt[:, 64:128])
                    qTp = ps.tile([64, 1024], BF16, tag="qTp")
                    nc.tensor.transpose(out=qTp[:, 0:128], in_=pt[:, 0:64],
                                        identity=ident[:])
                    nc.scalar.activation(q2T[:, i, :], qTp[:, 0:128],
                                         mybir.ActivationFunctionType.Copy)

            # ---- block loop (conv2 + final linear fused) ----
            with tc.tile_pool(name="sb", bufs=3) as sb, \
                 tc.tile_pool(name="sb2", bufs=2) as sb2, \
                 tc.tile_pool(name="tps", bufs=1, space="PSUM") as tps, \
                 tc.tile_pool(name="qps", bufs=1, space="PSUM") as qps, \
                 tc.tile_pool(name="aps", bufs=2, space="PSUM") as aps, \
                 tc.tile_pool(name="dps", bufs=2, space="PSUM") as dps:
                for i in range(B2):
                    ix = sb.tile([128, 128], I16, tag="ix")
                    nc.sync.dma_start(out=ix[:], in_=idx[i])
                    wt = sb.tile([128, 16], BF16, tag="wt")
                    nc.sync.dma_start(out=wt[:], in_=win[i])
                    xs = sb.tile([128, 16, 128], BF16, tag="xs")
                    nc.gpsimd.dma_gather(
                        xs[:, 0:8, :], tab[:, 0:128], ix[:, 0:64], 1024, 1024,
                        128, elem_step=256, single_packet=False,
                        queue_num=(2 * i) % 4)
                    nc.gpsimd.dma_gather(
                        xs[:, 8:16, :], _off(tab[:, 0:128], 128),
                        ix[:, 64:128], 1024, 1024, 128, elem_step=256,
                        single_packet=False, queue_num=(2 * i + 1) % 4)

                    agg = aps.tile([128, 512], F32, tag="agg")
                    for half in range(2):
                        kTp = tps.tile([64, 8, 128], BF16, tag="kTp")
                        for k in range(8):
                            t = half * 8 + k
                            nc.tensor.transpose(out=kTp[:, k, :],
                                                in_=xs[:, t, 0:64],
                                                identity=ident[:])
                        kT = sb2.tile([64, 8, 128], BF16, tag="kT")
                        nc.scalar.activation(kT[:], kTp[:],
                                             mybir.ActivationFunctionType.Copy)
                        alp = qps.tile([128, 8, 64], F32, tag="alp")
                        for k in range(8):
                            nc.tensor.matmul(
                                out=alp[:, k, 0:32], lhsT=kT[:, k, :],
                                rhs=q2T[:, i, (k // 2) * 32:(k // 2) * 32 + 32],
                                start=True, stop=True)
                        E = sb2.tile([128, 8, 32], BF16, tag="E")
                        nc.scalar.activation(E[:], alp[:, :, 0:32],
                                             mybir.ActivationFunctionType.Exp,
                                             scale=0.125)
                        M01 = sb2.tile([128, 8, 32], BF16, tag="M01")
                        nc.vector.tensor_tensor(
                            out=M01[:],
                            in0=wt[:, half * 8:(half + 1) * 8]
                                .unsqueeze(2).to_broadcast([128, 8, 32]),
                            in1=cpt[:].unsqueeze(1)
                                .to_broadcast([128, 8, 32]),
                            op=mybir.AluOpType.is_equal)
                        phi = sb2.tile([128, 8, 32], BF16, tag="phi")
                        nc.vector.tensor_mul(phi[:], E[:], M01[:])
                        for k in range(8):
                            t = half * 8 + k
                            w2 = k // 2
                            nc.tensor.matmul(
                                out=agg[w2 * 32:(w2 + 1) * 32, 0:64],
                                lhsT=phi[:, k, :], rhs=xs[:, t, 64:128],
                                start=(half == 0 and k % 2 == 0),
                                stop=False,
                                tile_position=(0, 32 * w2),
                                skip_group_check=True)
                            nc.tensor.matmul(
                                out=agg[w2 * 32:(w2 + 1) * 32, 64:65],
                                lhsT=phi[:, k, :], rhs=ones1[:],
                                start=False,
                                stop=(half == 1 and k % 2 == 1),
                                tile_position=(0, 32 * w2),
                                skip_group_check=True)

                    sc = sb.tile([128, 1], F32, tag="sc")
                    nc.vector.tensor_scalar_max(sc[:], agg[:, 64:65], 1e-30)
                    rs = sb.tile([128, 1], F32, tag="rs")
                    nc.vector.reciprocal(rs[:], sc[:])
                    h2 = sb.tile([128, 64], BF16, tag="h2")
                    nc.vector.tensor_mul(h2[:], agg[:, 0:64],
                                         rs[:].to_broadcast([128, 64]))
                    nc.vector.tensor_add(h2[:], h2[:], sk2S[:, i, :])
                    nc.scalar.activation(h2[:], h2[:],
                                         mybir.ActivationFunctionType.Relu)
                    hTp = dps.tile([64, 1024], BF16, tag="hTp")
                    nc.tensor.transpose(out=hTp[:, 0:128], in_=h2[:],
                                        identity=ident[:])
                    hT = sb.tile([64, 128], BF16, tag="hT")
                    nc.vector.tensor_copy(hT[:], hTp[:, 0:128])
                    op = dps.tile([128, 512], F32, tag="op")
                    nc.tensor.matmul(out=op[:, 0:2], lhsT=hT[:], rhs=Wlt[:],
                                     start=True, stop=True)
                    oo = sb.tile([128, 2], F32, tag="oo")
                    nc.vector.tensor_add(oo[:], op[:, 0:2], blt[:])
                    nc.sync.dma_start(out=outf[i * 128:(i + 1) * 128, :],
                                      in_=oo[:])
    nc.finalize()
    return nc


def _wrap_idx(v, n):
    """[n] int16 -> [128, n//16] dma_gather layout (16-wrap, 8x replicated)."""
    m = v.reshape(n // 16, 16).T
    return np.tile(m, (8, 1)).astype(np.int16)


def _prep(edge_index):
    """Sort edges by dst; pack nodes into 16-slot windows with per-parity
    capacity 128; 8 windows per iteration."""
    src = np.ascontiguousarray(edge_index[0]).astype(np.int64)
    dst = np.ascontiguousarray(edge_index[1]).astype(np.int64)
    E = src.shape[0]
    order = np.argsort(dst, kind="stable")
    s_sorted = src[order]
    d_sorted = dst[order]
    parity = (s_sorted & 1).astype(np.int64)
    deg = np.bincount(d_sorted, minlength=NNODE)
    dege = np.bincount(d_sorted[parity == 0], minlength=NNODE)
    dego = deg - dege
    cume = np.concatenate([[0], np.cumsum(deg)])
    targets = [round(E * c / NCORES) for c in range(1, NCORES)]
    nb = [0] + [int(np.searchsorted(cume, t)) for t in targets] + [NNODE]

    cores = []
    for c in range(NCORES):
        n0, n1 = nb[c], nb[c + 1]
        wbnd, na, ne, no, cnt = [], n0, 0, 0, 0
        for n in range(n0, n1):
            de, do = int(dege[n]), int(dego[n])
            assert de <= WCAP and do <= WCAP
            if cnt >= WS or ne + de > WCAP or no + do > WCAP:
                wbnd.append((na, n))
                na, ne, no, cnt = n, 0, 0, 0
            ne += de
            no += do
            cnt += 1
        if cnt:
            wbnd.append((na, n1))
        cores.append((n0, n1, wbnd))
    B2 = max((len(w) + NW - 1) // NW for _, _, w in cores)

    per_core = []
    for c in range(NCORES):
        n0, n1, wbnd = cores[c]
        idxw = np.zeros((B2, 128, 128), np.int16)
        winid = np.full((B2, 128, 16), -1.0, np.float32)
        slot_node = np.full((B2 * 128,), -1, np.int64)
        idxE = np.zeros((B2, NW * WCAP), np.int64)
        idxO = np.zeros((B2, NW * WCAP), np.int64)
        for widx, (na, nbd) in enumerate(wbnd):
            i, w = widx // NW, widx % NW
            nsl = nbd - na
            slot_node[i * 128 + w * WS:i * 128 + w * WS + nsl] = \
                np.arange(na, nbd)
            e0, e1 = int(cume[na]), int(cume[nbd])
            pa = parity[e0:e1]
            sid = (d_sorted[e0:e1] - na).astype(np.float32)
            se = s_sorted[e0:e1]
            ev, od = pa == 0, pa == 1
            nev, nod = int(ev.sum()), int(od.sum())
            idxE[i, w * WCAP:w * WCAP + nev] = se[ev] >> 1
            idxO[i, w * WCAP:w * WCAP + nod] = se[od] >> 1
            winid[i, :nev, w] = sid[ev]
            winid[i, :nod, 8 + w] = sid[od]
        for i in range(B2):
            idxw[i, :, 0:64] = _wrap_idx(idxE[i].astype(np.int16), 1024)
            idxw[i, :, 64:128] = _wrap_idx(idxO[i].astype(np.int16), 1024)
        win2 = winid.copy()
        for t in range(16):
            off = 16.0 * ((t % 8) % 2)
            v = win2[:, :, t] >= 0
            win2[:, :, t][v] += off
        per_core.append(dict(idx=idxw, win=winid.astype(BF),
                             win2=win2.astype(BF), slot_node=slot_node))
    return B2, per_core


def kernel(x, edge_index, Wq1, bq1, Wk1, bk1, Wv1, bv1, Ws1, bs1,
           Wq2, bq2, Wk2, bk2, Wv2, bv2, Ws2, bs2, Wl, bl):
    x = np.asarray(x, np.float32)
    B2, per_core = _prep(np.asarray(edge_index))
    NS = B2 * 128

    if ("L1", B2) not in _built:
        _built[("L1", B2)] = _build_L1(B2)
    if ("L2", B2) not in _built:
        _built[("L2", B2)] = _build_L2(B2)

    Wq1, Wk1 = np.asarray(Wq1, np.float32), np.asarray(Wk1, np.float32)
    Wv1, Ws1 = np.asarray(Wv1, np.float32), np.asarray(Ws1, np.float32)
    bq1, bv1, bs1 = (np.asarray(b, np.float32) for b in (bq1, bv1, bs1))
    Acat = np.zeros((64, 256), np.float32)
    bu = np.zeros((256,), np.float32)
    for h in range(4):
        sl = slice(h * 64, (h + 1) * 64)
        Acat[:, sl] = Wq1[:, sl] @ Wk1[:, sl].T
        bu[sl] = bq1[sl] @ Wk1[:, sl].T
    W1cat = np.concatenate([Acat, Ws1], axis=1)
    b1cat = np.concatenate([bu, bs1 + bv1])[None, :]
    W2cat = np.concatenate([Wk2, Wv2, Wq2, Ws2], axis=1).astype(np.float32)
    b2cat = np.concatenate([bk2, bv2, bq2, bs2])[None, :].astype(np.float32)

    tab1 = np.zeros((NPAIR, 128), BF)
    tab1[:NNODE // 2, 0:64] = x[0::2].astype(BF)
    tab1[:NNODE // 2, 64:128] = x[1::2].astype(BF)
    cpat1 = np.tile((np.arange(64) % 16).astype(BF)[None, :], (128, 1))
    cpat2 = np.tile(np.arange(32).astype(BF)[None, :], (128, 1))
    cids = list(range(NCORES))

    in1 = []
    for c in cids:
        pc = per_core[c]
        sn = pc["slot_node"]
        valid = sn >= 0
        xTc = np.zeros((64, NS), BF)
        xTc[:, valid] = x[sn[valid]].T.astype(BF)
        in1.append(dict(
            xT=xTc, W1=W1cat.astype(BF), b1=b1cat, tab=tab1,
            idx=pc["idx"], win=pc["win"], cpat=cpat1,
            Wv=Wv1.astype(BF), W2=W2cat.astype(BF), b2=b2cat,
            idn=np.eye(128, dtype=BF)))
    res1 = run_bass_kernel_spmd(_built[("L1", B2)], in1, cids)
    t1 = res1.exec_time_ns

    tab2 = np.zeros((NPAIR, 256), BF)
    p2s = []
    for c in cids:
        pc = per_core[c]
        sn = pc["slot_node"]
        valid = sn >= 0
        o = res1.results[c]["outt"]          # [NS, 256] bf16: k2|v2|q2|sk2
        kv = o[valid][:, 0:128]
        nodes = sn[valid]
        evn = (nodes & 1) == 0
        tab2[nodes[evn] >> 1, 0:128] = kv[evn]
        tab2[nodes[~evn] >> 1, 128:256] = kv[~evn]
        p2 = np.zeros((NS, 128), BF)
        p2[valid] = o[valid][:, 128:256]
        p2s.append(p2)

    in2 = []
    for c in cids:
        pc = per_core[c]
        in2.append(dict(
            p2=p2s[c], tab=tab2, idx=pc["idx"], win=pc["win2"], cpat=cpat2,
            Wl=np.asarray(Wl, np.float32).astype(BF),
            bl=np.asarray(bl, np.float32)[None, :],
            idn=np.eye(128, dtype=BF)))
    res2 = run_bass_kernel_spmd(_built[("L2", B2)], in2, cids)
    t2 = res2.exec_time_ns

    out = np.zeros((NNODE, 2), np.float32)
    for c in cids:
        pc = per_core[c]
        sn = pc["slot_node"]
        valid = sn >= 0
        out[sn[valid]] = res2.results[c]["outf"][valid]
    kernel.exec_times = (t1, t2)
    return out
